# revision 125
# baseline (speedup 1.0000x reference)
"""AttentionSubsample Trainium2 kernel: 8-core data-parallel over batch.

Layout strategy (per core, 4 batch elements):
  - All matmuls contract over the SBUF partition dim.
  - Matmul dtypes: the attention core and all x-dependent matmuls run bf16
    (full-rate PE at any moving dim; fp32 is 1/4 rate, f32r 1/4 rate below
    a 256-wide moving dim). Only the constant shp bias-row matmul stays
    float32r. PSUM accumulation is fp32 throughout.
  - The k-path BN shift is dropped entirely: it adds a per-query constant
    along the softmax axis, which cancels exactly in the normalization.
  - q is computed straight from a strided (grid-subsampled) view of the
    resident x tile - no separate subsampled input tensor.
  - v computed token-major [tok, feat] so attn@v needs no transpose; its BN
    shift is applied after attention via the softmax-denominator identity
    (an all-ones column per head gives the denominator as output row 64).
  - scores computed as s.T [ktok, qtok] per (head, chunk); bias applied as
    exp(s)*exp(bias) with a host-gathered bf16 exp-bias table.
  - softmax reciprocals taken directly from the attn@v PSUM denominator row,
    broadcast to 128 partitions with a small bf16 select matmul.
  - engine balance: ACT does exp + attention-output eviction; DVE does the
    other PSUM evictions, the bias-multiply (bf16 2x mode), reciprocals and
    the hardswish chain (4x tensor_scalar modes); Pool (gpsimd - no PSUM
    port, TensorTensor/TensorCopy/Memset only) takes memsets and a slice of
    the bias-multiplies.
  - software pipelining: attention pairs run a 3-stage pipeline (scores+exp
    / bias-mult+attn@v / evictions) emitted one pair apart so a pair's
    attn@v never head-of-line-blocks the next pair's scores in the in-order
    PE queue. g=0 traverses (head-pair, batch) as a wavefront with g=0/g=1
    k/v/q units spread across the windows; g=1 runs batch-outer with each
    pair's output-phase slice (reciprocal broadcast via a per-pair K=64
    select matmul + hardswish) flowing right behind its evictions.
"""

import sys

sys.path.insert(0, "/opt/trn_rl_repo")

from contextlib import ExitStack

import numpy as np
import ml_dtypes

import concourse.bass as bass
import concourse.tile as tile
from concourse import bacc
from concourse import mybir
from concourse.bass_utils import run_bass_kernel_spmd

F32 = mybir.dt.float32
F32R = mybir.dt.float32r
BF16 = mybir.dt.bfloat16
ALU = mybir.AluOpType
AF = mybir.ActivationFunctionType

B, N, NQ, IN, H, KD, D, OUT = 32, 784, 196, 384, 16, 32, 64, 512
HID, DH = 1536, 1024
RES, RES_, STRIDE = 28, 14, 2
SCALE = KD ** -0.5
EPS = 1e-5
NCORES = 8
BC = B // NCORES          # 4 batch elems per core
C, MC = 7, 112            # key-token chunks: 7 x 112 = 784
G, HG = 2, 8              # 2 head-groups of 8 heads

TRACE = False
LAST_RESULTS = None

_NC_CACHE = None


def _build_nc():
    nc = bacc.Bacc("TRN2", target_bir_lowering=False, debug=False,
                   num_devices=NCORES)

    xT = nc.dram_tensor("xT", [BC, IN, N], BF16, kind="ExternalInput").ap()
    wk = nc.dram_tensor("wk", [IN, 512], BF16, kind="ExternalInput").ap()
    wv = nc.dram_tensor("wv", [IN, DH], BF16, kind="ExternalInput").ap()
    wq = nc.dram_tensor("wq", [IN, 512], BF16, kind="ExternalInput").ap()
    wp = nc.dram_tensor("wp", [DH, OUT], BF16, kind="ExternalInput").ap()
    shq = nc.dram_tensor("shq", [128, 4], F32, kind="ExternalInput").ap()
    shv = nc.dram_tensor("shv", [128, 8], F32, kind="ExternalInput").ap()
    shp = nc.dram_tensor("shp", [1, OUT], F32R, kind="ExternalInput").ap()
    ebias = nc.dram_tensor("ebias", [MC, H, C, NQ], BF16,
                           kind="ExternalInput").ap()
    seld = nc.dram_tensor("seld", [64, 128], BF16, kind="ExternalInput").ap()
    out = nc.dram_tensor("out", [BC, NQ, OUT], F32, kind="ExternalOutput").ap()

    with tile.TileContext(nc) as tc, ExitStack() as ctx:
        singles = ctx.enter_context(tc.tile_pool(name="singles", bufs=1))
        biasp = ctx.enter_context(tc.tile_pool(name="biasp", bufs=4))
        kqp = ctx.enter_context(tc.tile_pool(name="kqp", bufs=2))
        vp = ctx.enter_context(tc.tile_pool(name="vp", bufs=2))
        texpp = ctx.enter_context(tc.tile_pool(name="texpp", bufs=5))
        tmpp = ctx.enter_context(tc.tile_pool(name="tmpp", bufs=3))
        finp = ctx.enter_context(tc.tile_pool(name="finp", bufs=2))
        mmp = ctx.enter_context(tc.tile_pool(name="mmp", bufs=2, space="PSUM"))
        scp = ctx.enter_context(tc.tile_pool(name="scp", bufs=2, space="PSUM"))
        opp = ctx.enter_context(tc.tile_pool(name="opp", bufs=2, space="PSUM"))

        # --- persistent SBUF; load order tuned so the first k matmuls of
        # batch elem 0 can start after ~2 small DMAs ---
        wk_sb = singles.tile([128, 3, 512], BF16)
        wv_sb = singles.tile([128, 3, DH], BF16)
        wq_sb = singles.tile([128, 3, 512], BF16)
        wp_sb = singles.tile([128, 8, OUT], BF16)
        xtb = [singles.tile([128, 3, N], BF16, name=f"xtb{b}")
               for b in range(BC)]
        xTr = xT.rearrange("b (c p) n -> b p c n", p=128)
        shq_sb = singles.tile([128, 4], F32)
        # x0 rides the ACT DMA queue in parallel with the weight loads on
        # the SP queue, so the first k matmuls' inputs land ~2us sooner
        for kk in range(3):
            nc.scalar.dma_start(xtb[0][:, kk, :], xTr[0, :, kk, :])
            nc.sync.dma_start(wk_sb[:, kk, :],
                              wk.rearrange("(c p) n -> p c n", p=128)[:, kk, :])
        nc.sync.dma_start(wq_sb, wq.rearrange("(c p) n -> p c n", p=128))
        nc.sync.dma_start(shq_sb, shq)
        for kk in range(3):
            nc.sync.dma_start(wv_sb[:, kk, :],
                              wv.rearrange("(c p) n -> p c n", p=128)[:, kk, :])
        for kk in range(3):
            nc.sync.dma_start(xtb[1][:, kk, :], xTr[1, :, kk, :])
        # first two head-pairs' bias tables load early, before the
        # non-critical x/constant loads, all on the SP queue: the ACT queue
        # carries only x0 so the exp stream can start decoding immediately
        bias_t = {}
        for hp in range(3):
            bias_t[hp] = biasp.tile([MC, 2, C, NQ], BF16, tag="bias",
                                    name="bias_g")
            nc.sync.dma_start(bias_t[hp], ebias[:, 2 * hp:2 * hp + 2, :, :])
        for b in range(2, BC):
            for kk in range(3):
                nc.sync.dma_start(xtb[b][:, kk, :], xTr[b, :, kk, :])
        shv_sb = singles.tile([128, 8], F32)
        nc.sync.dma_start(shv_sb, shv)
        shp_sb = singles.tile([1, OUT], F32R)
        nc.sync.dma_start(shp_sb, shp)
        # sel[p, m] = 1 iff row m of a feature tile belongs to the pair's
        # head at partition p (p=0 -> rows 0-63, p=32 -> rows 64-127; other
        # partitions zero). A K=64 matmul against rec64[b][:, t, :]
        # broadcasts the pair's softmax reciprocals. Engine writes must
        # start at a 32-aligned partition, hence the 0/32 placement.
        sel = singles.tile([64, 128], BF16, name="sel")
        nc.sync.dma_start(sel, seld)
        nc.sync.dma_start(wp_sb, wp.rearrange("(c p) n -> p c n", p=128))
        ones1 = singles.tile([1, 128], F32R)
        nc.gpsimd.memset(ones1.bitcast(F32), 1.0)
        # preload the exp activation table during the DMA-bound startup
        dummy = singles.tile([1, 8], F32)
        nc.scalar.activation(dummy, ones1.bitcast(F32)[0:1, 0:8], AF.Exp)

        acc = [singles.tile([128, 8, NQ], BF16, name=f"acc{b}")
               for b in range(BC)]
        # per-pair softmax reciprocals: partition 32*j (head-within-pair),
        # free = (feature tile t, q); zero-filled once so the unwritten
        # partitions can't feed NaN*0 into the select matmul
        rec64 = []
        for b in range(BC):
            r64 = singles.tile([64, 8, NQ], BF16, name=f"rec64{b}")
            nc.gpsimd.memset(r64, 0.0)
            rec64.append(r64)
        hsw_all = [singles.tile([128, 8, NQ], BF16, name=f"hsw{b}")
                   for b in range(BC)]

        def kvq_kq(g, b):
            """k and q for head-group g, batch elem b (scores inputs)."""
            k_sb = kqp.tile([128, 2, N], BF16, tag=f"k{b}", name=f"k{g}{b}")
            q_sb = kqp.tile([128, 2, NQ], BF16, tag=f"q{b}", name=f"q{g}{b}")
            xg = xtb[b].rearrange("p c (r s) -> p c r s", r=RES)
            for m2 in range(2):
                for n2 in range(2):
                    pk = mmp.tile([128, 512], F32, tag="mm", name="pk")
                    for kk in range(3):
                        nc.tensor.matmul(
                            pk[:, :392],
                            lhsT=wk_sb[:, kk, 256 * g + 128 * m2:
                                       256 * g + 128 * m2 + 128],
                            rhs=xtb[b][:, kk, 392 * n2:392 * n2 + 392],
                            start=(kk == 0), stop=(kk == 2))
                    nc.vector.tensor_copy(
                        k_sb[:, m2, 392 * n2:392 * n2 + 392], pk[:, :392])
                pq = mmp.tile([128, 512], F32, tag="mm", name="pq")
                for kk in range(3):
                    nc.tensor.matmul(
                        pq[:, :NQ],
                        lhsT=wq_sb[:, kk, 256 * g + 128 * m2:
                                   256 * g + 128 * m2 + 128],
                        rhs=xg[:, kk, ::STRIDE, ::STRIDE],
                        start=(kk == 0), stop=(kk == 2))
                nc.vector.tensor_scalar_add(
                    q_sb[:, m2, :], pq[:, :NQ],
                    shq_sb[:, 2 * g + m2:2 * g + m2 + 1])
            return k_sb, q_sb

        def kvq_v(g, b, cs=range(C), vtp=None):
            """v token-major (512 features of group g), with an all-ones
            column appended per head for the softmax denominator."""
            if vtp is None:
                vtp = vp.tile([MC, C, 8, 65], BF16, tag=f"v{b}",
                              name=f"v{g}{b}")
                nc.gpsimd.memset(vtp[:, :, :, 64:65], 1.0)
            for c in cs:
                pv = mmp.tile([128, 512], F32, tag="mm", name="pv")
                for kk in range(3):
                    nc.tensor.matmul(
                        pv[:MC, :],
                        lhsT=xtb[b][:, kk, MC * c:MC * c + MC],
                        rhs=wv_sb[:, kk, 512 * g:512 * g + 512],
                        start=(kk == 0), stop=(kk == 2))
                nc.vector.tensor_copy(
                    vtp[:, c, :, 0:64],
                    pv[:MC, :].rearrange("p (h d) -> p h d", d=64))
            return vtp

        def kvq(g, b):
            k_sb, q_sb = kvq_kq(g, b)
            vtp = kvq_v(g, b)
            return k_sb, vtp, q_sb

        p2 = [None]  # stage2 (bias-mult + attn@v) of the previous pair
        p3 = [None]  # stage3 (evictions) of the pair before that

        def attention(g, hp, b, bias_g, tiles, pump_now=True):
            """one head-pair of attention, 3-stage software pipeline:
            stage1 (scores+exp) emitted now; stage2 (bias-mult+attn@v) of
            the PREVIOUS pair and stage3 (evictions) of the pair before it
            are emitted behind it, so a pair's attn@v never head-of-line
            blocks the next pair's scores on the in-order PE queue."""
            hhs = (2 * hp, 2 * hp + 1)
            k_sb, vtp, q_sb = tiles
            texp2 = [texpp.tile([MC, C, NQ], BF16, tag="texp",
                                name=f"texp{hh}") for hh in hhs]
            for cq, cs in ((0, (0, 1, 2, 3)), (1, (4, 5, 6))):
                sc2 = [scp.tile([MC, 2, 512], F32, tag="sc",
                                name=f"sc{hh}") for hh in hhs]
                for ci, c in enumerate(cs):
                    for j, hh in enumerate(hhs):
                        pb = 32 * (hh % 4)
                        m2 = hh // 4
                        nc.tensor.matmul(
                            sc2[j][:, ci // 2,
                                   196 * (ci % 2):196 * (ci % 2) + 196],
                            lhsT=k_sb[pb:pb + 32, m2, MC * c:MC * c + MC],
                            rhs=q_sb[pb:pb + 32, m2, :],
                            start=True, stop=True,
                            tile_position=(pb, 0),
                            skip_group_check=True)
                for j, hh in enumerate(hhs):
                    texp, sc = texp2[j], sc2[j]
                    if cq == 0:
                        nc.scalar.activation(
                            texp[:, 0:4, :].rearrange(
                                "p (a b) q -> p a b q", b=2),
                            sc[:, :, 0:392].rearrange(
                                "p a (b q) -> p a b q", q=196), AF.Exp)
                    else:
                        nc.scalar.activation(
                            texp[:, 4:6, :],
                            sc[:, 0, 0:392].rearrange(
                                "p (a q) -> p a q", q=196), AF.Exp)
                        nc.scalar.activation(texp[:, 6, :],
                                             sc[:, 1, 0:196], AF.Exp)

            def stage2(g=g, hp=hp, b=b, bias_g=bias_g, vtp=vtp, texp2=texp2,
                       hhs=hhs):
                # a callable vtp resolves lazily: its tile may be emitted
                # after this pair's stage1 but before stage2 (one pair later)
                if callable(vtp):
                    vtp = vtp()
                # bias multiply: one bf16 2x-mode DVE op per head (a slice
                # goes to the Pool engine, which legally only does SBUF
                # TensorTensor/TensorCopy/Memset)
                for j, hh in enumerate(hhs):
                    eng = nc.gpsimd if (b == 0 and j == 1) else nc.vector
                    eng.tensor_tensor(texp2[j][:, :, :], texp2[j][:, :, :],
                                      bias_g[:, j, :, :], ALU.mult)
                # attn @ v (+ denominator row) per head of the pair
                op = opp.tile([65, 2, NQ], F32, tag="op")
                for j, hh in enumerate(hhs):
                    for c in range(C):
                        nc.tensor.matmul(
                            op[:, j, :],
                            lhsT=vtp[:, c, hh, :],
                            rhs=texp2[j][:, c, :],
                            start=(c == 0), stop=(c == C - 1))
                t = 4 * g + hp

                def stage3(op=op, b=b, t=t):
                    # both ACT copies before the DVE reciprocals: the pool's
                    # ordered-reader bookkeeping otherwise chains the second
                    # copy behind the first reciprocal across engines
                    for j in range(2):
                        nc.scalar.activation(
                            acc[b][64 * j:64 * j + 64, t, :],
                            op[0:64, j, :], AF.Copy)
                    for j in range(2):
                        # softmax reciprocal from the denominator row
                        with nc.allow_low_precision(reason="bf16 recips"):
                            nc.vector.reciprocal(
                                rec64[b][32 * j:32 * j + 1, t, :],
                                op[64:65, j, :])
                    # g=1 pairs emit their output-phase slice right here
                    # (depends only on this pair's acc rows + recips); g=0
                    # slices are deferred into the g=1 windows, which have
                    # PE/DVE slack
                    if t >= 4:
                        out_chain(b, [t])
                    if t == 7:
                        out_proj(b)
                return stage3

            if not pump_now:
                return stage2
            pump(stage2)

        def pump(stage2):
            if p2[0] is not None:
                s3 = p2[0]()
                if p3[0] is not None:
                    p3[0]()
                p3[0] = s3
            p2[0] = stage2

        def flush_pairs():
            if p2[0] is not None:
                s3 = p2[0]()
                p2[0] = None
                if p3[0] is not None:
                    p3[0]()
                s3()
                p3[0] = None
            elif p3[0] is not None:
                p3[0]()
                p3[0] = None

        def out_chain(b, ts):
            """normalize + hardswish for feature-tiles `ts` of batch b."""
            hsw = hsw_all[b]
            for t in ts:
                rep = mmp.tile([128, 512], F32, tag="mm", name="rep")
                nc.tensor.matmul(rep[:, :NQ], lhsT=sel,
                                 rhs=rec64[b][:, t, :], start=True, stop=True)
                # v_t = acc*recip + shift ; hsw6 = v_t * clamp(v_t+3,0,6)
                t1 = tmpp.tile([128, NQ], BF16, tag="t1")
                nc.vector.tensor_tensor(t1, acc[b][:, t, :], rep[:, :NQ],
                                        ALU.mult)
                vv = tmpp.tile([128, NQ], BF16, tag="vv")
                nc.vector.tensor_scalar_add(vv, t1, shv_sb[:, t:t + 1])
                t3 = tmpp.tile([128, NQ], BF16, tag="t1")
                nc.vector.tensor_scalar(t3, vv, -3.0, 3.0, ALU.max, ALU.min)
                nc.vector.scalar_tensor_tensor(hsw[:, t, :], t3, 3.0, vv,
                                               ALU.add, ALU.mult)

        def out_proj(b):
            """projection + store for batch elem b."""
            hsw = hsw_all[b]
            for mt, msz in ((0, 128), (1, 68)):
                po = mmp.tile([128, 512], F32, tag="mm", name="po")
                nc.tensor.matmul(po[:msz, :], lhsT=ones1[0:1, 0:msz],
                                 rhs=shp_sb, start=True, stop=False,
                                 skip_group_check=True)
                for kk in range(8):
                    nc.tensor.matmul(
                        po[:msz, :],
                        lhsT=hsw[:, kk, 128 * mt:128 * mt + msz],
                        rhs=wp_sb[:, kk, :], start=False,
                        stop=(kk == 7), skip_group_check=True)
                fin = finp.tile([128, OUT], F32, tag="fin")
                nc.scalar.activation(fin[:msz, :], po[:msz, :], AF.Copy)
                # out stores go on the ACT DMA queue so they never
                # head-of-line-block the ebias loads on the SP queue
                nc.scalar.dma_start(out[b, 128 * mt:128 * mt + msz, :],
                                    fin[:msz, :])

        # ---- emission: software-pipelined phases ----
        # kvq units are spread so the PE always has dense independent work
        # while ACT grinds through the exp stream: b0's k/q up front (its v
        # right after the first pair's scores), the other g=0 units inside
        # the first head-pair window, g=1's units one per subsequent window.
        g0, g1 = {}, {}
        kq0 = kvq_kq(0, 0)
        v0 = {}
        g0[0] = (kq0[0], (lambda: v0[0]), kq0[1])

        g1kq = {}
        # wavefront traversal of (hp, b): pairs on diagonal s = hp + b run
        # together, so the kvq units spread over several windows instead of
        # clumping while ACT idles
        for s in range(7):
            for hp in range(min(s, 3), -1, -1):
                b = s - hp
                if b >= BC:
                    continue
                if hp not in bias_t:
                    bias_t[hp] = biasp.tile([MC, 2, C, NQ], BF16, tag="bias", name="bias_g")
                    nc.sync.dma_start(bias_t[hp],
                                      ebias[:, 2 * hp:2 * hp + 2, :, :])
                if hp == 0 and b >= 1:
                    g0[b] = kvq(0, b)
                if hp == 1 and b == 0:
                    # pair (1,0): its scores slot in between the two halves
                    # of unit (0,0)'s v-matmuls
                    st2 = attention(0, hp, b, bias_t[hp], g0[b],
                                    pump_now=False)
                    kvq_v(0, 0, cs=range(4, C), vtp=v0[0])
                    pump(st2)
                else:
                    attention(0, hp, b, bias_t[hp], g0[b])
                if hp == 0 and b == 0:
                    v0[0] = kvq_v(0, 0, cs=range(0, 4))
                if hp == 3 and b <= 2:
                    g1kq[b] = kvq_kq(1, b)
                    g1[b] = (g1kq[b][0], kvq_v(1, b), g1kq[b][1])

        # g=1 attention, batch-elem outer; each pair's output-phase slice
        # flows with its deferred stage3, and the g=0 slices (t = hp) are
        # sprinkled one per pair into this batch elem's own window
        for b in range(BC):
            for hp in range(4):
                if b == 0 and hp == 0:
                    g1kq[3] = kvq_kq(1, 3)
                if b == 0 and hp == 1:
                    g1[3] = (g1kq[3][0], kvq_v(1, 3), g1kq[3][1])
                bias_g = biasp.tile([MC, 2, C, NQ], BF16, tag="bias")
                nc.sync.dma_start(bias_g,
                                  ebias[:, 8 + 2 * hp:8 + 2 * hp + 2, :, :])
                attention(1, hp, b, bias_g, g1[b])
                out_chain(b, [hp])
        flush_pairs()
    nc.compile()
    return nc


def _prepare_in_maps(inputs):
    inp = {k: np.asarray(v) for k, v in inputs.items()}
    x = inp["x"].astype(np.float32)          # [32, 784, 384]
    Wkv, Wq, Wp = inp["Wkv"], inp["Wq"], inp["Wp"]
    biases, idxs = inp["biases"], inp["idxs"].astype(np.int64)
    bf16 = ml_dtypes.bfloat16

    s_kv = inp["kv_w"] / np.sqrt(inp["kv_var"] + EPS)
    wkv = (Wkv * s_kv[:, None]).astype(np.float32)
    sh_kv = (inp["kv_b"] - inp["kv_mean"] * s_kv).astype(np.float32)
    wkv3 = wkv.reshape(H, KD + D, IN)
    sh3 = sh_kv.reshape(H, KD + D)
    wkT = np.ascontiguousarray(wkv3[:, :KD, :].reshape(H * KD, IN).T)
    wvT = np.ascontiguousarray(wkv3[:, KD:, :].reshape(H * D, IN).T)
    sh_v = np.ascontiguousarray(sh3[:, KD:].reshape(H * D))

    s_q = inp["q_w"] / np.sqrt(inp["q_var"] + EPS)
    wqT = np.ascontiguousarray((Wq * (s_q * SCALE)[:, None]).T)
    sh_q = ((inp["q_b"] - inp["q_mean"] * s_q) * SCALE).astype(np.float32)

    s_p = inp["p_w"] / np.sqrt(inp["p_var"] + EPS)
    wpT = np.ascontiguousarray(((Wp * s_p[:, None]) / 6.0).T)
    sh_p = (inp["p_b"] - inp["p_mean"] * s_p).astype(np.float32)

    eb = np.exp(biases.astype(np.float64))[:, idxs]      # [16, 196, 784]
    eb = eb.transpose(0, 2, 1).reshape(H, C, MC, NQ)
    eb = np.ascontiguousarray(eb.transpose(2, 0, 1, 3)).astype(bf16)

    shq_h = np.ascontiguousarray(sh_q.reshape(4, 128).T)
    shv_h = np.ascontiguousarray(sh_v.reshape(8, 128).T)
    shp_h = np.ascontiguousarray(sh_p.reshape(1, OUT))

    sel_h = np.zeros((64, 128), bf16)
    sel_h[0, 0:64] = 1.0
    sel_h[32, 64:128] = 1.0
    shared = {"wk": wkT.astype(bf16), "wv": wvT.astype(bf16),
              "wq": wqT.astype(bf16), "wp": wpT.astype(bf16),
              "shq": shq_h, "shv": shv_h, "shp": shp_h, "ebias": eb,
              "seld": sel_h}
    in_maps = []
    for i in range(NCORES):
        xb = x[BC * i:BC * i + BC]
        m = dict(shared)
        m["xT"] = np.ascontiguousarray(xb.transpose(0, 2, 1)).astype(bf16)
        in_maps.append(m)
    return in_maps


def kernel(**inputs):
    global _NC_CACHE, LAST_RESULTS
    in_maps = _prepare_in_maps(inputs)
    if _NC_CACHE is None:
        _NC_CACHE = _build_nc()
    res = run_bass_kernel_spmd(_NC_CACHE, in_maps,
                               core_ids=list(range(NCORES)), trace=TRACE)
    LAST_RESULTS = res
    return np.concatenate([res.results[i]["out"] for i in range(NCORES)],
                          axis=0)
